# revision 1
# baseline (speedup 1.0000x reference)
"""CrossInteraction kernel for TRN2, 8-core data parallel.

Math: interaction[b,i,j] = x1[b,i] * x2[b,j]
  mean_dim1[b,i] = x1[b,i] * mean_j(x2[b,j])
  mean_dim2[b,j] = x2[b,j] * mean_i(x1[b,i])
  out = concat([mean_dim1, mean_dim2], axis=1)   # (B, DIM1+DIM2)

The (B, DIM1, DIM2) interaction tensor is never materialized: per batch row
we need one row-mean of x1, one row-mean of x2, and two scaled copies.

Sharding: pure data parallel over batch — 256 rows / 8 cores = 32 rows/core.

Layout: each per-core tensor (32, F) is loaded as a [128, F/4] SBUF tile
with partition = 32*c + b (c = feature-chunk 0..3, b = batch row). This
fills all 128 partitions (full SBUF DMA port bandwidth, 4x DVE lanes).
Row sums are finished with partition-shifted adds that leave the per-row
sum replicated across all 4 chunks' partitions, so the final
tensor_scalar broadcast needs no further shuffling.
"""

import numpy as np

import concourse.bass as bass
import concourse.bacc as bacc
import concourse.tile as tile
from concourse import mybir
from concourse.bass_utils import run_bass_kernel_spmd

BATCH, DIM1, DIM2 = 256, 512, 1024
N_CORES = 8
B_LOC = BATCH // N_CORES  # 32 rows per core
F1 = DIM1 // 4  # 128
F2 = DIM2 // 4  # 256

_FP32 = mybir.dt.float32


def build_nc() -> bass.Bass:
    nc = bacc.Bacc(
        "TRN2", target_bir_lowering=False, debug=False, num_devices=N_CORES
    )
    x1 = nc.dram_tensor("x1", [B_LOC, DIM1], _FP32, kind="ExternalInput").ap()
    x2 = nc.dram_tensor("x2", [B_LOC, DIM2], _FP32, kind="ExternalInput").ap()
    out = nc.dram_tensor("out", [B_LOC, DIM1 + DIM2], _FP32, kind="ExternalOutput").ap()

    # DRAM views matching the [128, F/4] partition=32c+b SBUF layout.
    # DMA pairs src/dst elements in flat enumeration order, so a 3D
    # (c, b, f) DRAM view against a [128, F/4] SBUF tile lands row b's
    # chunk c at partition 32c+b.
    x1_v = x1.rearrange("b (c f) -> c b f", c=4)
    x2_v = x2.rearrange("b (c f) -> c b f", c=4)
    o1_v = out[:, :DIM1].rearrange("b (c f) -> c b f", c=4)
    o2_v = out[:, DIM1:].rearrange("b (c f) -> c b f", c=4)

    with tile.TileContext(nc) as tc:
        with tc.tile_pool(name="p", bufs=1) as pool:
            x1_t = pool.tile([128, F1], _FP32)
            x2_t = pool.tile([128, F2], _FP32)
            nc.sync.dma_start(x1_t[:], x1_v)
            nc.scalar.dma_start(x2_t[:], x2_v)

            # q[:,0] = partial row-sums of x1, q[:,1] = of x2 (per chunk)
            q = pool.tile([128, 2], _FP32)
            nc.vector.reduce_sum(q[:, 0:1], x1_t[:], axis=mybir.AxisListType.X)
            nc.vector.reduce_sum(q[:, 1:2], x2_t[:], axis=mybir.AxisListType.X)

            # Fold the 4 chunk groups (partition p = 32c+b) down to full
            # row sums at partitions 0..31, then replicate back to all 128.
            # Two-SBUF-input ops must share a base partition, so each fold
            # is a partition-shifted copy followed by an aligned add.
            t1 = pool.tile([64, 2], _FP32)
            nc.vector.tensor_copy(t1[:, :], q[64:128, :])
            a = pool.tile([64, 2], _FP32)
            nc.vector.tensor_add(a[:, :], q[0:64, :], t1[:, :])
            t2 = pool.tile([32, 2], _FP32)
            nc.vector.tensor_copy(t2[:, :], a[32:64, :])
            brd = pool.tile([128, 2], _FP32)
            nc.vector.tensor_add(brd[0:32, :], a[0:32, :], t2[:, :])
            nc.vector.tensor_copy(brd[32:64, :], brd[0:32, :])
            nc.vector.tensor_copy(brd[64:128, :], brd[0:64, :])

            # o1 = x1 * mean(x2) ; o2 = x2 * mean(x1)
            o1 = pool.tile([128, F1], _FP32)
            o2 = pool.tile([128, F2], _FP32)
            nc.vector.tensor_scalar(
                o1[:], x1_t[:], brd[:, 1:2], 1.0 / DIM2,
                mybir.AluOpType.mult, mybir.AluOpType.mult,
            )
            nc.vector.tensor_scalar(
                o2[:], x2_t[:], brd[:, 0:1], 1.0 / DIM1,
                mybir.AluOpType.mult, mybir.AluOpType.mult,
            )
            nc.sync.dma_start(o1_v, o1[:])
            nc.scalar.dma_start(o2_v, o2[:])
    nc.compile()
    return nc


def run(x1: np.ndarray, x2: np.ndarray, trace: bool = False):
    """Build + run on 8 cores; returns (full_output, BassKernelResults)."""
    nc = build_nc()
    x1 = np.ascontiguousarray(np.asarray(x1, dtype=np.float32))
    x2 = np.ascontiguousarray(np.asarray(x2, dtype=np.float32))
    in_maps = [
        {
            "x1": x1[i * B_LOC:(i + 1) * B_LOC],
            "x2": x2[i * B_LOC:(i + 1) * B_LOC],
        }
        for i in range(N_CORES)
    ]
    res = run_bass_kernel_spmd(nc, in_maps, list(range(N_CORES)), trace=trace)
    full = np.concatenate([r["out"] for r in res.results], axis=0)
    return full, res


def kernel(x1: np.ndarray, x2: np.ndarray) -> np.ndarray:
    full, _ = run(x1, x2, trace=False)
    return full

